# revision 1
# baseline (speedup 1.0000x reference)
"""Grouped linear (MoE routed GEMM) on 8 Trainium2 NeuronCores.

out[t] = hidden_states[t] @ weight[g(t)] where g(t) is the expert owning
token t (contiguous groups sized by tokens_per_expert).

Strategy (expert-parallel, token-balanced):
  - All group sizes are multiples of 128 -> 64 row-tiles of 128 tokens;
    each core gets exactly 8 row-tiles (1024 tokens). SPMD static slot
    pattern [0,0,0,1,1,1,2,2]: 3 weight slots per core covering 3/3/2
    row-tiles; the host decomposes the per-expert tile counts into
    sixteen 3-tile parts + eight 2-tile parts, assigns (expert ->
    core,slot), and packs per-core inputs in exact consume order.
  - The ENTIRE HBM schedule rides ONE HWDGE ring (scalar engine, whose
    framework preamble retires earliest), in exact consume order:
    8 wave-0 batches [xt_k rt0-2 | w0_k] (352KB bf16 -- only what
    slot 0 eats, so even a cold-ramping DMA hose paces it), the
    remaining xt row-tiles, wv1 in 4 chunks, wv2 in 4 chunks, then the
    16 output stores (queued behind the load tail, exactly as the
    ridge requires). One FIFO ring delivers in order at full HBM rate
    (~358GB/s/core): no cross-queue round-robin starvation, no ladder,
    one per-transfer semaphore, one wait per slot-0 round. Per-engine
    descriptor FIFO makes a later transfer's semaphore imply all
    earlier ones, so downstream gates stay sparse. (SWDGE int8+cast
    variants were tried and rejected: an active SWDGE stream starves
    HWDGE rings to ~100GB/s, and total SBUF-port bytes don't drop.)
  - PE: slot 0 k-major (6 chains advance per landing batch), slot 1 as
    two 3-chain k-major waves (paced by the wv1 chunks; only 3 PSUM
    banks are free at its start), slot 2 hybrid: 2 chains k-major
    riding the wv2 tail, then chain-major, with the very last chain as
    two sequential 256-wide half-chains on two DIFFERENT PSUM banks so
    the first half's cast+store overlap the second half's matmuls --
    the end-of-kernel serial tail (cast+store+receipt) nearly halves.
    16 junk warmup matmuls (N=256, ~213ns each cold) on uninitialized
    SBUF bridge the ~3.4us HAM clock-gate ramp so the first real MM
    runs at the full 2.4GHz rate right as batch 0 lands.
  - Per-chain output slices (no buffer reuse, no store-wait sems).
    The final >=256 store quiesce is mandatory: ending the program
    with DMAs in flight wedges the device (NRT_EXEC_UNIT_UNRECOVERABLE).

Measured (core 0 NTFF): 43.9-45.9us vs 52.2us baseline. The exec
window = [first framework memset .. last teardown instruction]: it
includes a fixed ~7.7us NKI wrapper epilogue (zeroes all 256 sems) and
excludes ~6us of preamble. Controllable part is ~2us from the ridge
floor (10.5MB HBM traffic ~ 29.3us ~ 27.65us bf16 PE roofline + edges);
run-to-run spread ~+-1.5us tracks the DMA hose's cold-start ramp.
"""

import os
import numpy as np
import ml_dtypes

from concourse import bacc, mybir
from concourse.bass_utils import run_bass_kernel_spmd

T, D, G, NCORES = 8192, 1024, 8, 8
TPC = T // NCORES            # tokens per core
RT = TPC // 128              # row tiles per core (8)
KT = D // 128                # contraction tiles (8)
PATTERN = (0, 0, 0, 1, 1, 1, 2, 2)   # row-tile -> weight slot
WARMUP_MMS = int(os.environ.get("K_WARMUP", "17"))

CDT = mybir.dt.bfloat16      # compute dtype on device
NP_CDT = ml_dtypes.bfloat16
ODT = mybir.dt.bfloat16      # device output dtype (host upcasts)

_PROG = None
LAST_RESULTS = None          # test harness reads exec_time_ns from here


def _build_program():
    """Raw (no-Tile) program, identical on all 8 cores.

    DRAM inputs, host-packed in consume order:
      b0, b1 [128, 2048] bf16: wave-0 batches k=0,1 (xt_k | w0_k),
                               issued ahead of the Block entry barrier
      wv0 [KT-2, 128, 2048] bf16: wave-0 batches k=2..7
      wv1 [128, KT*1024] bf16: slot-1 weight, k-tile k at cols k*1024
      wv2 [128, KT*1024] bf16: slot-2 weight, likewise
    """
    nc = bacc.Bacc("TRN2", target_bir_lowering=False, debug=False,
                   num_devices=NCORES)
    # wave-0 batch k = [xt_k cols 0-383 (slot-0 row tiles) | w0_k]:
    # only what slot 0 consumes, so a cold DMA hose still paces it.
    # The remaining xt columns (row tiles 3-7) follow as xtr.
    BW = 384 + 1024
    # batches 0-2 ship in two pieces each: [xt_k rt0-2 | w0_k oh0]
    # unblocks that round's first three chains ~0.35us sooner under a
    # cold-ramping hose; [w0_k oh1] follows in-ring.
    ba_d = nc.dram_tensor("ba", [4, 128, 896], CDT, kind="ExternalInput")
    bb_d = nc.dram_tensor("bb", [4, 128, 512], CDT, kind="ExternalInput")
    wv0_d = nc.dram_tensor("wv0", [KT - 4, 128, BW], CDT,
                           kind="ExternalInput")
    xtra_d = nc.dram_tensor("xtra", [128, KT * 384], CDT,
                           kind="ExternalInput")
    xtrb_d = nc.dram_tensor("xtrb", [128, KT * 256], CDT,
                           kind="ExternalInput")
    wv1_d = nc.dram_tensor("wv1", [128, KT * 1024], CDT,
                           kind="ExternalInput")
    wv2_d = nc.dram_tensor("wv2", [128, KT * 1024], CDT,
                           kind="ExternalInput")
    o_d = nc.dram_tensor("o", [TPC, D], ODT, kind="ExternalOutput")

    # batch k in SBUF: cols [k*BW, (k+1)*BW) = xt_k(rt0-2) | w0_k;
    # xtra k-tile k at cols [k*384,..) = xt_k(rt3-5) (slot-1 rows);
    # xtrb k-tile k at cols [k*256,..) = xt_k(rt6-7) (slot-2 rows,
    # shipped after wv1 so the waveA-critical ring prefix is shorter)
    b_sb = nc.alloc_sbuf_tensor("bs", [128, KT * BW], CDT).ap()
    xtra_sb = nc.alloc_sbuf_tensor("xtras", [128, KT * 384], CDT).ap()
    xtrb_sb = nc.alloc_sbuf_tensor("xtrbs", [128, KT * 256], CDT).ap()
    wv1_sb = nc.alloc_sbuf_tensor("wv1s", [128, KT * 1024], CDT).ap()
    wv2_sb = nc.alloc_sbuf_tensor("wv2s", [128, KT * 1024], CDT).ap()
    ot_sb = nc.alloc_sbuf_tensor("ots", [128, 16 * 512], ODT).ap()
    warm_sb = nc.alloc_sbuf_tensor("warm", [128, 512], CDT).ap()
    psum = [nc.alloc_psum_tensor(f"ps{i}", [128, 512], mybir.dt.float32).ap()
            for i in range(8)]

    # Per-transfer DMA sems: a shared counting sem is unsound with
    # multiple transfers in flight on one ring (per-engine incs from a
    # later transfer can reach 16*k while an earlier one is pending).
    s_b = [nc.alloc_semaphore(f"sb{k}") for k in range(KT)]
    s_bb = [nc.alloc_semaphore(f"sbb{k}") for k in range(4)]
    s_xr = [nc.alloc_semaphore(f"sxr{j}") for j in range(2)]
    s_w1 = [nc.alloc_semaphore(f"sw1_{j}") for j in range(4)]
    s_w2 = [nc.alloc_semaphore(f"sw2_{j}") for j in range(4)]
    s_mm = nc.alloc_semaphore("smm")   # chain stop completions
    s_cp = nc.alloc_semaphore("scp")   # PSUM->SBUF cast completions
    s_st = nc.alloc_semaphore("sst")   # store completions (total count)

    # chain c = (rt, oh): rt = c//2, oh = c%2; completion order == c.
    # banks: slot-0 chains 0-5 -> 0-5; slot-1 wave A (6,7,8) -> 6,7,0;
    # wave B (9,10,11) -> 1,2,3; slot-2 (12..15) -> 4,5,6,7.
    # Warmup also uses bank 6 (in-order PE frees it before chain 6).
    bank_of = [0, 1, 2, 3, 4, 5, 6, 7, 0, 1, 2, 3, 4, 5, 6, 7]

    def xt_ap(k, rt):
        if rt < 3:
            lo = k * BW + rt * 128
            return b_sb[:, lo: lo + 128]
        if rt < 6:
            lo = k * 384 + (rt - 3) * 128
            return xtra_sb[:, lo: lo + 128]
        lo = k * 256 + (rt - 6) * 128
        return xtrb_sb[:, lo: lo + 128]

    def w_ap(s, k, oh):
        if s == 0:
            lo = k * BW + 384 + oh * 512
            return b_sb[:, lo: lo + 512]
        t = wv1_sb if s == 1 else wv2_sb
        return t[:, k * 1024 + oh * 512: k * 1024 + (oh + 1) * 512]

    # Everything is emitted into the pre-barrier main block: each
    # engine's stream is purely semaphore-driven, so no engine ever
    # waits on a Block entry barrier before starting (sems are zeroed
    # by the prior run's wrapper epilogue, and the wrapper has its own
    # terminal rendezvous). The ENTIRE HBM schedule -- 16 loads in
    # consume order, then 16 stores -- rides the single scalar-engine
    # HWDGE ring: the scalar engine's framework preamble retires
    # earliest (~0.9us before sync's), one FIFO ring delivers in order
    # at full HBM rate with no cross-queue round-robin, and stores
    # naturally queue behind the load tail, exactly as the ridge
    # requires.

    sc = nc.scalar
    for k in range(4):
        sc.dma_start(b_sb[:, k * BW: k * BW + 896],
                     ba_d[k]).then_inc(s_b[k], 16)
        sc.dma_start(b_sb[:, k * BW + 896:(k + 1) * BW],
                     bb_d[k]).then_inc(s_bb[k], 16)
    for k in range(4, KT):
        sc.dma_start(b_sb[:, k * BW:(k + 1) * BW],
                     wv0_d[k - 4]).then_inc(s_b[k], 16)
    # xt row tiles 3-7 (two chunks) interleaved with the wv1 chunks in
    # waveA's consume order: each xtr chunk precedes the wv1 chunk
    # whose round-gate needs it, so per-engine FIFO makes the existing
    # wv1 gates cover xtr with no extra waits.
    def xtra_chunk(j):
        sc.dma_start(xtra_sb[:, j * 1536:(j + 1) * 1536],
                     xtra_d[:, j * 1536:(j + 1) * 1536]).then_inc(s_xr[j], 16)

    def wv_chunk(sems, src, dst, j):
        sc.dma_start(dst[:, j * 2048:(j + 1) * 2048],
                     src[:, j * 2048:(j + 1) * 2048]).then_inc(sems[j], 16)

    xtra_chunk(0)
    wv_chunk(s_w1, wv1_d, wv1_sb, 0)
    wv_chunk(s_w1, wv1_d, wv1_sb, 1)
    xtra_chunk(1)
    wv_chunk(s_w1, wv1_d, wv1_sb, 2)
    wv_chunk(s_w1, wv1_d, wv1_sb, 3)
    # xtrb halves interleave with the first wv2 chunks: slot-2's
    # early rounds unblock 256KB sooner; the round-4 gate (s_w2[2])
    # implies xtrb-b by FIFO.
    sc.dma_start(xtrb_sb[:, 0:1024], xtrb_d[:, 0:1024]).then_inc(s_xr[0], 16)
    wv_chunk(s_w2, wv2_d, wv2_sb, 0)
    wv_chunk(s_w2, wv2_d, wv2_sb, 1)
    sc.dma_start(xtrb_sb[:, 1024:2048],
                 xtrb_d[:, 1024:2048]).then_inc(s_xr[1], 16)
    wv_chunk(s_w2, wv2_d, wv2_sb, 2)
    wv_chunk(s_w2, wv2_d, wv2_sb, 3)
    for c in range(15):
        rt, oh = c // 2, c % 2
        sc.wait_ge(s_cp, c + 1)
        sc.dma_start(
            o_d[rt * 128:(rt + 1) * 128, oh * 512:(oh + 1) * 512],
            ot_sb[:, c * 512:(c + 1) * 512]).then_inc(s_st, 16)
    for h in range(2):
        lo = 15 * 512 + h * 256
        sc.wait_ge(s_cp, 16 + h)
        sc.dma_start(
            o_d[896:1024, 512 + h * 256:768 + h * 256],
            ot_sb[:, lo: lo + 256]).then_inc(s_st, 16)

    # -- tensor: junk warmups on uninitialized SBUF bridge the HAM
    # clock-gate ramp until batch 0 lands (~3.4us of sustained PE
    # activity flips the gate right as data arrives). The PSUM target
    # is overwritten by the first start=True MM of its real tenant.
    te = nc.tensor
    for _ in range(WARMUP_MMS):
        te.matmul(psum[6][:, 0:256], warm_sb[:, 0:128], warm_sb[:, 0:256],
                  start=True, stop=True)
    # slot 0: k-major; one inline wait per round (xt_k and w0_k share
    # batch k's transfer). Rounds 0-2 run the oh0 chains first so they
    # start on the smaller a-piece; the oh1 chains gate on the b-piece.
    for k in range(KT):
        order = (0, 2, 4, 1, 3, 5) if k < 4 else range(6)
        for ci in order:
            rt, oh = ci // 2, ci % 2
            mm = te.matmul(psum[ci][:], xt_ap(k, rt), w_ap(0, k, oh),
                           start=(k == 0), stop=(k == KT - 1))
            if ci == 0:
                mm._wait_ge(s_b[k], 16)
            elif k < 4 and ci == 1:
                mm._wait_ge(s_bb[k], 16)
            if k == KT - 1:
                mm.then_inc(s_mm)
    # slot 1: two k-major waves of 3 chains (only banks 6,7,0 resp.
    # 1,2,3 are free in time); paced by the wv1 chunks. (A 4+2 wave
    # split was tried to slow waveA's wv1 consumption to delivery rate,
    # but it crashed the device -- unresolved; 3+3 is the proven form.)
    for wave, chains in ((0, (6, 7, 8)), (1, (9, 10, 11))):
        for k in range(KT):
            for c in chains:
                rt, oh = c // 2, c % 2
                mm = te.matmul(psum[bank_of[c]][:], xt_ap(k, rt),
                               w_ap(1, k, oh),
                               start=(k == 0), stop=(k == KT - 1))
                if wave == 0 and k % 2 == 0 and c == 6:
                    mm._wait_ge(s_w1[k // 2], 16)
                if k == 0 and c >= 8:
                    # bank reused: prior tenant's cast done
                    mm._wait_ge(s_cp, c - 7)
                if k == KT - 1:
                    mm.then_inc(s_mm)
    # slot 2 hybrid: chains 12-13 k-major (paced by the wv2 chunks, so
    # compute overlaps the load-stream tail in hose-limited runs), then
    # chains 14-15 chain-major (staggered stops keep the final tail one
    # cast+store+receipt deep in compute-limited runs).
    for k in range(KT):
        if k % 2 == 0:
            te.wait_ge(s_w2[k // 2], 16)
        for c in (12, 13):
            rt, oh = c // 2, c % 2
            mm = te.matmul(psum[bank_of[c]][:], xt_ap(k, rt),
                           w_ap(2, k, oh),
                           start=(k == 0), stop=(k == KT - 1))
            if k == 0:
                mm._wait_ge(s_cp, c - 7)      # bank free
            if k == KT - 1:
                mm.then_inc(s_mm)
    for k in range(KT):
        mm = te.matmul(psum[bank_of[14]][:], xt_ap(k, 7), w_ap(2, k, 0),
                       start=(k == 0), stop=(k == KT - 1))
        if k == 0:
            mm._wait_ge(s_cp, 7)              # bank free
        if k == KT - 1:
            mm.then_inc(s_mm)
    # The very last chain runs as two sequential 256-wide half-chains
    # on two DIFFERENT banks (7 then 0, both long free), so the first
    # half's cast+store overlap the second half's matmuls and the
    # end-of-kernel serial tail halves. (Same-bank splitting crashes:
    # PE-write + DVE-read of one bank is illegal.)
    for h, bank in ((0, 7), (1, 0)):
        for k in range(KT):
            lo = k * 1024 + 512 + h * 256
            mm = te.matmul(psum[bank][:, 0:256],
                           xt_ap(k, 7), wv2_sb[:, lo: lo + 256],
                           start=(k == 0), stop=(k == KT - 1))
            if k == 0:
                mm._wait_ge(s_cp, 8 + h)      # bank free (c7 / chain 8)
            if k == KT - 1:
                mm.then_inc(s_mm)

    # -- vector: PSUM->SBUF casts in chain-completion order
    for c in range(15):
        cp = nc.vector.tensor_copy(ot_sb[:, c * 512:(c + 1) * 512],
                                   psum[bank_of[c]][:])
        cp._wait_ge(s_mm, c + 1)
        cp.then_inc(s_cp)
    for h, bank in ((0, 7), (1, 0)):
        lo = 15 * 512 + h * 256
        cp = nc.vector.tensor_copy(ot_sb[:, lo: lo + 256],
                                   psum[bank][:, 0:256])
        cp._wait_ge(s_mm, 16 + h)
        cp.then_inc(s_cp)

    # -- sync: quiesce (all stores landed) before the final rendezvous.
    # Required: ending the program with DMAs in flight wedges the
    # device (NRT_EXEC_UNIT_UNRECOVERABLE).
    nc.sync.wait_ge(s_st, 16 * 17)

    with nc.Block():
        pass

    nc.compile()
    return nc


def _get_program():
    global _PROG
    if _PROG is None:
        _PROG = _build_program()
    return _PROG


def _solve_parts(tiles_per_expert):
    """Decompose per-expert tile counts into 16 parts of 3 tiles and 8
    parts of 2 tiles. Returns (threes, twos) as lists of expert ids, or
    None if infeasible."""
    t = list(tiles_per_expert)
    f = [c % 2 for c in t]              # number of 3-parts per expert
    if any(3 * f[g] > t[g] for g in range(len(t))):
        return None
    h = [(t[g] - 3 * f[g]) // 2 for g in range(len(t))]
    # each f+=2 converts three 2-parts into two 3-parts
    while sum(h) > 8:
        g = max(range(len(t)), key=lambda i: h[i])
        if h[g] < 3:
            return None
        f[g] += 2
        h[g] -= 3
    if sum(h) != 8 or sum(f) != 16:
        return None
    threes, twos = [], []
    for g in range(len(t)):
        threes += [g] * f[g]
        twos += [g] * h[g]
    return threes, twos


def _numpy_fallback(hidden_states, weight, counts):
    out = np.empty((hidden_states.shape[0], weight.shape[2]), np.float32)
    start = 0
    for g in range(weight.shape[0]):
        end = start + int(counts[g])
        out[start:end] = hidden_states[start:end].astype(np.float32) @ \
            weight[g].astype(np.float32)
        start = end
    return out


def kernel(hidden_states, weight, tokens_per_expert):
    counts = np.asarray(tokens_per_expert).astype(np.int64)
    out_dtype = hidden_states.dtype

    ok = (hidden_states.shape == (T, D) and weight.shape == (G, D, D)
          and counts.shape == (G,) and counts.sum() == T
          and np.all(counts % 128 == 0) and np.all(counts >= 0))
    parts = _solve_parts(counts // 128) if ok else None
    if parts is None:
        return _numpy_fallback(hidden_states, weight, counts).astype(out_dtype)
    threes, twos = parts

    # Global preprocessing: transpose+cast activations once, cast weights.
    ht = np.ascontiguousarray(
        np.asarray(hidden_states, dtype=np.float32).astype(NP_CDT).T)
    wc = np.asarray(weight, dtype=np.float32).astype(NP_CDT)  # [G, D, D]

    # Per-expert global row offsets; consume tiles in order.
    expert_row = dict(
        (g, int(o)) for g, o in enumerate(np.concatenate(
            [[0], np.cumsum(counts)[:-1]])))

    in_maps = []
    core_rows = []       # per core: list of (global_row_start, n_rows)
    for c in range(NCORES):
        part_list = [(threes[2 * c], 3 * 128), (threes[2 * c + 1], 3 * 128),
                     (twos[c], 2 * 128)]
        spans = []
        for g, nrows in part_list:
            r0 = expert_row[g]
            expert_row[g] = r0 + nrows
            spans.append((r0, nrows))
        core_rows.append(spans)
        # xt_c: [D, TPC] activations (pre-transposed); k-tile k = rows
        # k*128..k*128+127.
        xt_c = np.concatenate(
            [ht[:, r0:r0 + n] for r0, n in spans], axis=1)
        w_slots = [wc[g] for g, _ in part_list]   # 3 x [D, D] bf16

        # wave-0 batch k packs slot-0's slice of k-tile k, partition-
        # major: batch[k, p] = xt[k*128+p, 0:384] | w0[k*128+p, :];
        # xtr[p, k*640:(k+1)*640] = xt[k*128+p, 384:1024]
        xt_k = xt_c.reshape(KT, 128, TPC)
        w0_k = w_slots[0].reshape(KT, 128, D)
        wv0 = np.empty((KT, 128, 384 + 1024), dtype=NP_CDT)
        wv0[:, :, 0:384] = xt_k[:, :, 0:384]
        wv0[:, :, 384:1408] = w0_k
        xtra = np.ascontiguousarray(
            xt_k[:, :, 384:768].transpose(1, 0, 2).reshape(128, KT * 384))
        xtrb = np.ascontiguousarray(
            xt_k[:, :, 768:1024].transpose(1, 0, 2).reshape(128, KT * 256))
        # wv1/wv2 [128, KT*1024]: row p = concat_k W[k*128+p, :]
        wv1 = np.ascontiguousarray(
            w_slots[1].reshape(KT, 128, D).transpose(1, 0, 2).reshape(
                128, KT * D))
        wv2 = np.ascontiguousarray(
            w_slots[2].reshape(KT, 128, D).transpose(1, 0, 2).reshape(
                128, KT * D))
        in_maps.append({"ba": np.ascontiguousarray(wv0[0:4, :, 0:896]),
                        "bb": np.ascontiguousarray(wv0[0:4, :, 896:1408]),
                        "wv0": np.ascontiguousarray(wv0[4:]),
                        "xtra": xtra, "xtrb": xtrb,
                        "wv1": wv1, "wv2": wv2})

    nc = _get_program()
    global LAST_RESULTS
    LAST_RESULTS = run_bass_kernel_spmd(nc, in_maps, list(range(NCORES)))

    out = np.empty((T, D), np.float32)
    for c in range(NCORES):
        o_c = np.asarray(LAST_RESULTS.results[c]["o"]).astype(np.float32)
        r = 0
        for r0, n in core_rows[c]:
            out[r0:r0 + n] = o_c[r:r + n]
            r += n
    return out.astype(out_dtype, copy=False)



# revision 3
# speedup vs baseline: 1.0157x; 1.0157x over previous
"""Grouped linear (MoE routed GEMM) on 8 Trainium2 NeuronCores.

out[t] = hidden_states[t] @ weight[g(t)] where g(t) is the expert owning
token t (contiguous groups sized by tokens_per_expert).

Strategy (expert-parallel, token-balanced):
  - All group sizes are multiples of 128 -> 64 row-tiles of 128 tokens;
    each core gets exactly 8 row-tiles (1024 tokens). SPMD static slot
    pattern [0,0,0,1,1,1,2,2]: 3 weight slots per core covering 3/3/2
    row-tiles; the host decomposes the per-expert tile counts into
    sixteen 3-tile parts + eight 2-tile parts, assigns (expert ->
    core,slot), and packs per-core inputs in exact consume order.
  - All loads ride ONE HWDGE ring (scalar engine, whose framework
    preamble retires earliest) in exact consume order. Wave-0 (slot-0
    activations+weights) is packed for MAXIMUM DMA LINE SIZE: batches
    k=0,1 as single [128,1408] transfers (2816B lines) and k=2..7 as
    three [128,2816] PAIRS (5632B lines). NTFF analysis of the split
    896/512-col layout showed the wave-0 phase ran at ~230GB/s
    (1792/1024B lines are packet-rate limited) vs 400-430GB/s for
    4096B lines; big lines compress the wave-0 prefix ~4us and pull
    wv1/wv2 forward, closing the slot-1 start stall.
  - PE: 13 junk warmup matmuls (N=256 on uninitialized SBUF) bridge
    the clock-gate ramp until batch 0 lands (~9.3us); the HAM clock
    flip tracks DMA-start+3.3us, so real MMs issued before it run at
    half clock - starting them as early as possible retires more of
    them cheaply. Slot 0 k-major (6 chains, one gate per round: b0,
    b1, then one per pair on even k). Slot 1 as two 3-chain k-major
    waves paced by the wv1 chunks. Slot 2 CHAIN-major (loads finish
    ~28.5us, well before slot 2 runs, so there is no load tail to
    ride): chains 12,13,14 stop early and spread their casts+stores,
    and the last chain runs as two sequential 256-wide half-chains on
    two DIFFERENT PSUM banks so the first half's cast+store overlap
    the second half's matmuls.
  - Stores: row-tile PAIRED [128,1024] stores (2048B lines) for rt0-6
    gated on both halves' casts; rt7 stays split (512 + 256 + 256)
    for tail overlap, and the FINAL 256-wide store rides the SYNC
    engine's HWDGE ring (warmed by a tiny dummy store at program
    start) so it does not queue behind the scalar ring's previous
    store. Final quiesce (all stores landed) is mandatory: ending the
    program with DMAs in flight wedges the device.

Measured baseline of the split-layout version: 46.3us graded window,
of which ~7.5us is the fixed NKI wrapper epilogue (zeroes all 256
sems) and ~0.5us preamble tail. This layout targets ~41-42us.
"""

import os
import numpy as np
import ml_dtypes

from concourse import bacc, mybir
from concourse.bass_utils import run_bass_kernel_spmd

T, D, G, NCORES = 8192, 1024, 8, 8
TPC = T // NCORES            # tokens per core
RT = TPC // 128              # row tiles per core (8)
KT = D // 128                # contraction tiles (8)
PATTERN = (0, 0, 0, 1, 1, 1, 2, 2)   # row-tile -> weight slot
WARMUP_MMS = int(os.environ.get("K_WARMUP", "13"))

CDT = mybir.dt.bfloat16      # compute dtype on device
NP_CDT = ml_dtypes.bfloat16
ODT = mybir.dt.bfloat16      # device output dtype (host upcasts)

BW = 384 + 1024              # batch cols: [xt_k rt0-2 | w0_k]

_PROG = None
LAST_RESULTS = None          # test harness reads exec_time_ns from here


def _build_program():
    """Raw (no-Tile) program, identical on all 8 cores.

    DRAM inputs, host-packed in consume order:
      b01 [2, 128, 1408] bf16: wave-0 batches k=0,1 (xt_k rt0-2 | w0_k)
      bp  [3, 128, 2816] bf16: wave-0 batch pairs (2,3),(4,5),(6,7) -
                               5632B DMA lines
      xtra [128, KT*384] bf16: slot-1 row-tile activations (rt3-5)
      xtrb [128, KT*256] bf16: slot-2 row-tile activations (rt6-7)
      wv1, wv2 [128, KT*1024] bf16: slot-1/2 weights, k-tile k at
                               cols k*1024 (4096B lines)
    """
    nc = bacc.Bacc("TRN2", target_bir_lowering=False, debug=False,
                   num_devices=NCORES)
    b01_d = nc.dram_tensor("b01", [2, 128, BW], CDT, kind="ExternalInput")
    bp_d = nc.dram_tensor("bp", [3, 128, 2 * BW], CDT, kind="ExternalInput")
    xtra_d = nc.dram_tensor("xtra", [128, KT * 384], CDT,
                            kind="ExternalInput")
    xtrb_d = nc.dram_tensor("xtrb", [128, KT * 256], CDT,
                            kind="ExternalInput")
    wv1_d = nc.dram_tensor("wv1", [128, KT * 1024], CDT,
                           kind="ExternalInput")
    wv2_d = nc.dram_tensor("wv2", [128, KT * 1024], CDT,
                           kind="ExternalInput")
    o_d = nc.dram_tensor("o", [TPC, D], ODT, kind="ExternalOutput")
    scr_d = nc.dram_tensor("scr", [128, 8], CDT, kind="ExternalOutput")

    # batch k in SBUF: cols [k*BW, (k+1)*BW) = xt_k(rt0-2) | w0_k
    b_sb = nc.alloc_sbuf_tensor("bs", [128, KT * BW], CDT).ap()
    xtra_sb = nc.alloc_sbuf_tensor("xtras", [128, KT * 384], CDT).ap()
    xtrb_sb = nc.alloc_sbuf_tensor("xtrbs", [128, KT * 256], CDT).ap()
    wv1_sb = nc.alloc_sbuf_tensor("wv1s", [128, KT * 1024], CDT).ap()
    wv2_sb = nc.alloc_sbuf_tensor("wv2s", [128, KT * 1024], CDT).ap()
    ot_sb = nc.alloc_sbuf_tensor("ots", [128, 16 * 512], ODT).ap()
    warm_sb = nc.alloc_sbuf_tensor("warm", [128, 512], CDT).ap()
    psum = [nc.alloc_psum_tensor(f"ps{i}", [128, 512], mybir.dt.float32).ap()
            for i in range(8)]

    # Per-transfer DMA sems: a shared counting sem is unsound with
    # multiple transfers in flight on one ring.
    s_b0 = nc.alloc_semaphore("sb0")
    s_b1 = nc.alloc_semaphore("sb1")
    s_p = [nc.alloc_semaphore(f"sp{j}") for j in range(3)]
    s_xr = [nc.alloc_semaphore(f"sxr{j}") for j in range(2)]
    s_w1 = [nc.alloc_semaphore(f"sw1_{j}") for j in range(4)]
    s_w2 = [nc.alloc_semaphore(f"sw2_{j}") for j in range(4)]
    s_mm = nc.alloc_semaphore("smm")   # chain stop completions
    s_cp = nc.alloc_semaphore("scp")   # PSUM->SBUF cast completions
    s_st = nc.alloc_semaphore("sst")   # store completions (total count)

    # chain c = (rt, oh): rt = c//2, oh = c%2; completion order == c.
    # banks: slot-0 chains 0-5 -> 0-5; slot-1 wave A (6,7,8) -> 6,7,0;
    # wave B (9,10,11) -> 1,2,3; slot-2 (12..15) -> 4,5,6,7.
    # Warmup also uses bank 6 (in-order PE frees it before chain 6).
    bank_of = [0, 1, 2, 3, 4, 5, 6, 7, 0, 1, 2, 3, 4, 5, 6, 7]

    def xt_ap(k, rt):
        if rt < 3:
            lo = k * BW + rt * 128
            return b_sb[:, lo: lo + 128]
        if rt < 6:
            lo = k * 384 + (rt - 3) * 128
            return xtra_sb[:, lo: lo + 128]
        lo = k * 256 + (rt - 6) * 128
        return xtrb_sb[:, lo: lo + 128]

    def w_ap(s, k, oh):
        if s == 0:
            lo = k * BW + 384 + oh * 512
            return b_sb[:, lo: lo + 512]
        t = wv1_sb if s == 1 else wv2_sb
        return t[:, k * 1024 + oh * 512: k * 1024 + (oh + 1) * 512]

    # Everything is emitted into the pre-barrier main block: each
    # engine's stream is purely semaphore-driven.

    sc = nc.scalar
    sc.dma_start(b_sb[:, 0:BW], b01_d[0]).then_inc(s_b0, 16)
    sc.dma_start(b_sb[:, BW:2 * BW], b01_d[1]).then_inc(s_b1, 16)
    for j in range(3):
        sc.dma_start(b_sb[:, (2 + 2 * j) * BW:(4 + 2 * j) * BW],
                     bp_d[j]).then_inc(s_p[j], 16)

    def xtra_chunk(j):
        sc.dma_start(xtra_sb[:, j * 1536:(j + 1) * 1536],
                     xtra_d[:, j * 1536:(j + 1) * 1536]).then_inc(s_xr[j], 16)

    def wv_chunk(sems, src, dst, j):
        sc.dma_start(dst[:, j * 2048:(j + 1) * 2048],
                     src[:, j * 2048:(j + 1) * 2048]).then_inc(sems[j], 16)

    # xtr chunks precede the wv chunk whose round-gate needs them, so
    # per-engine FIFO makes the wv gates cover xtr with no extra waits.
    xtra_chunk(0)
    wv_chunk(s_w1, wv1_d, wv1_sb, 0)
    wv_chunk(s_w1, wv1_d, wv1_sb, 1)
    xtra_chunk(1)
    wv_chunk(s_w1, wv1_d, wv1_sb, 2)
    wv_chunk(s_w1, wv1_d, wv1_sb, 3)
    sc.dma_start(xtrb_sb[:, 0:1024], xtrb_d[:, 0:1024]).then_inc(s_xr[0], 16)
    wv_chunk(s_w2, wv2_d, wv2_sb, 0)
    wv_chunk(s_w2, wv2_d, wv2_sb, 1)
    sc.dma_start(xtrb_sb[:, 1024:2048],
                 xtrb_d[:, 1024:2048]).then_inc(s_xr[1], 16)
    wv_chunk(s_w2, wv2_d, wv2_sb, 2)
    wv_chunk(s_w2, wv2_d, wv2_sb, 3)
    # Paired [128,1024] stores (2048B lines) for rt0-6: both halves'
    # casts done -> one store. rt7 split for tail overlap.
    for rt in range(7):
        sc.wait_ge(s_cp, 2 * rt + 2)
        sc.dma_start(o_d[rt * 128:(rt + 1) * 128, :],
                     ot_sb[:, rt * 1024:(rt + 1) * 1024]).then_inc(s_st, 16)
    sc.wait_ge(s_cp, 15)
    sc.dma_start(o_d[896:1024, 0:512],
                 ot_sb[:, 14 * 512:15 * 512]).then_inc(s_st, 16)
    sc.wait_ge(s_cp, 16)
    sc.dma_start(o_d[896:1024, 512:768],
                 ot_sb[:, 15 * 512:15 * 512 + 256]).then_inc(s_st, 16)

    # -- sync engine: second HWDGE ring for the very last store, so it
    # overlaps the scalar ring's previous store instead of queueing
    # behind it. A tiny dummy store at stream start warms the ring.
    sy = nc.sync
    sy.dma_start(scr_d[:, :], warm_sb[:, 0:8]).then_inc(s_st, 16)
    sy.wait_ge(s_cp, 17)
    sy.dma_start(o_d[896:1024, 768:1024],
                 ot_sb[:, 15 * 512 + 256:16 * 512]).then_inc(s_st, 16)
    # quiesce (all 11 transfers with s_st landed) before teardown.
    sy.wait_ge(s_st, 16 * 11)

    # -- tensor: junk warmups on uninitialized SBUF bridge the clock
    # ramp until batch 0 lands. The PSUM target is overwritten by the
    # first start=True MM of its real tenant.
    te = nc.tensor
    for _ in range(WARMUP_MMS):
        te.matmul(psum[6][:, 0:256], warm_sb[:, 0:128], warm_sb[:, 0:256],
                  start=True, stop=True)
    # slot 0: k-major; one inline gate per round on the first chain.
    for k in range(KT):
        for ci in range(6):
            rt, oh = ci // 2, ci % 2
            mm = te.matmul(psum[ci][:], xt_ap(k, rt), w_ap(0, k, oh),
                           start=(k == 0), stop=(k == KT - 1))
            if ci == 0:
                if k == 0:
                    mm._wait_ge(s_b0, 16)
                elif k == 1:
                    mm._wait_ge(s_b1, 16)
                elif k % 2 == 0:
                    mm._wait_ge(s_p[k // 2 - 1], 16)
            if k == KT - 1:
                mm.then_inc(s_mm)
    # slot 1: two k-major waves of 3 chains (only banks 6,7,0 resp.
    # 1,2,3 are free in time); paced by the wv1 chunks.
    for wave, chains in ((0, (6, 7, 8)), (1, (9, 10, 11))):
        for k in range(KT):
            for c in chains:
                rt, oh = c // 2, c % 2
                mm = te.matmul(psum[bank_of[c]][:], xt_ap(k, rt),
                               w_ap(1, k, oh),
                               start=(k == 0), stop=(k == KT - 1))
                if wave == 0 and k % 2 == 0 and c == 6:
                    mm._wait_ge(s_w1[k // 2], 16)
                if k == 0 and c >= 8:
                    # bank reused: prior tenant's cast done
                    mm._wait_ge(s_cp, c - 7)
                if k == KT - 1:
                    mm.then_inc(s_mm)
    # slot 2 chain-major: loads finish well before slot 2 runs, so
    # chains 12,13,14 stop early and spread their casts+stores across
    # the remaining compute. Chain 12 carries the wv2 chunk gates
    # (in-order PE covers chains 13,14).
    for c in (12, 13, 14):
        rt, oh = c // 2, c % 2
        for k in range(KT):
            if c == 12 and k % 2 == 0:
                te.wait_ge(s_w2[k // 2], 16)
            mm = te.matmul(psum[bank_of[c]][:], xt_ap(k, rt),
                           w_ap(2, k, oh),
                           start=(k == 0), stop=(k == KT - 1))
            if k == 0:
                mm._wait_ge(s_cp, c - 7)      # bank free
            if k == KT - 1:
                mm.then_inc(s_mm)
    # The very last chain runs as two sequential 256-wide half-chains
    # on two DIFFERENT banks (7 then 0, both long free), so the first
    # half's cast+store overlap the second half's matmuls. (Same-bank
    # splitting crashes: PE-write + DVE-read of one bank is illegal.)
    for h, bank in ((0, 7), (1, 0)):
        for k in range(KT):
            lo = k * 1024 + 512 + h * 256
            mm = te.matmul(psum[bank][:, 0:256],
                           xt_ap(k, 7), wv2_sb[:, lo: lo + 256],
                           start=(k == 0), stop=(k == KT - 1))
            if k == 0:
                mm._wait_ge(s_cp, 8 + h)      # bank free (chain 7 / 8)
            if k == KT - 1:
                mm.then_inc(s_mm)

    # -- vector: PSUM->SBUF casts in chain-completion order
    for c in range(15):
        cp = nc.vector.tensor_copy(ot_sb[:, c * 512:(c + 1) * 512],
                                   psum[bank_of[c]][:])
        cp._wait_ge(s_mm, c + 1)
        cp.then_inc(s_cp)
    for h, bank in ((0, 7), (1, 0)):
        lo = 15 * 512 + h * 256
        cp = nc.vector.tensor_copy(ot_sb[:, lo: lo + 256],
                                   psum[bank][:, 0:256])
        cp._wait_ge(s_mm, 16 + h)
        cp.then_inc(s_cp)

    with nc.Block():
        pass

    nc.compile()
    return nc


def _get_program():
    global _PROG
    if _PROG is None:
        _PROG = _build_program()
    return _PROG


def _solve_parts(tiles_per_expert):
    """Decompose per-expert tile counts into 16 parts of 3 tiles and 8
    parts of 2 tiles. Returns (threes, twos) as lists of expert ids, or
    None if infeasible."""
    t = list(tiles_per_expert)
    f = [c % 2 for c in t]              # number of 3-parts per expert
    if any(3 * f[g] > t[g] for g in range(len(t))):
        return None
    h = [(t[g] - 3 * f[g]) // 2 for g in range(len(t))]
    # each f+=2 converts three 2-parts into two 3-parts
    while sum(h) > 8:
        g = max(range(len(t)), key=lambda i: h[i])
        if h[g] < 3:
            return None
        f[g] += 2
        h[g] -= 3
    if sum(h) != 8 or sum(f) != 16:
        return None
    threes, twos = [], []
    for g in range(len(t)):
        threes += [g] * f[g]
        twos += [g] * h[g]
    return threes, twos


def _numpy_fallback(hidden_states, weight, counts):
    out = np.empty((hidden_states.shape[0], weight.shape[2]), np.float32)
    start = 0
    for g in range(weight.shape[0]):
        end = start + int(counts[g])
        out[start:end] = hidden_states[start:end].astype(np.float32) @ \
            weight[g].astype(np.float32)
        start = end
    return out


def kernel(hidden_states, weight, tokens_per_expert):
    counts = np.asarray(tokens_per_expert).astype(np.int64)
    out_dtype = hidden_states.dtype

    ok = (hidden_states.shape == (T, D) and weight.shape == (G, D, D)
          and counts.shape == (G,) and counts.sum() == T
          and np.all(counts % 128 == 0) and np.all(counts >= 0))
    parts = _solve_parts(counts // 128) if ok else None
    if parts is None:
        return _numpy_fallback(hidden_states, weight, counts).astype(out_dtype)
    threes, twos = parts

    # Global preprocessing: transpose+cast activations once, cast weights.
    ht = np.ascontiguousarray(
        np.asarray(hidden_states, dtype=np.float32).astype(NP_CDT).T)
    wc = np.asarray(weight, dtype=np.float32).astype(NP_CDT)  # [G, D, D]

    # Per-expert global row offsets; consume tiles in order.
    expert_row = dict(
        (g, int(o)) for g, o in enumerate(np.concatenate(
            [[0], np.cumsum(counts)[:-1]])))

    in_maps = []
    core_rows = []       # per core: list of (global_row_start, n_rows)
    for c in range(NCORES):
        part_list = [(threes[2 * c], 3 * 128), (threes[2 * c + 1], 3 * 128),
                     (twos[c], 2 * 128)]
        spans = []
        for g, nrows in part_list:
            r0 = expert_row[g]
            expert_row[g] = r0 + nrows
            spans.append((r0, nrows))
        core_rows.append(spans)
        # xt_c: [D, TPC] activations (pre-transposed); k-tile k = rows
        # k*128..k*128+127.
        xt_c = np.concatenate(
            [ht[:, r0:r0 + n] for r0, n in spans], axis=1)
        w_slots = [wc[g] for g, _ in part_list]   # 3 x [D, D] bf16

        # wave-0 batch k, partition-major: batch[k, p] =
        # xt[k*128+p, 0:384] | w0[k*128+p, :]  -> [KT, 128, 1408]
        xt_k = xt_c.reshape(KT, 128, TPC)
        w0_k = w_slots[0].reshape(KT, 128, D)
        wv0 = np.empty((KT, 128, BW), dtype=NP_CDT)
        wv0[:, :, 0:384] = xt_k[:, :, 0:384]
        wv0[:, :, 384:BW] = w0_k
        b01 = np.ascontiguousarray(wv0[0:2])
        # pairs (2,3),(4,5),(6,7): row p = [batch2j(p,:)|batch2j+1(p,:)]
        bp = np.ascontiguousarray(
            wv0[2:].reshape(3, 2, 128, BW).transpose(0, 2, 1, 3).reshape(
                3, 128, 2 * BW))
        xtra = np.ascontiguousarray(
            xt_k[:, :, 384:768].transpose(1, 0, 2).reshape(128, KT * 384))
        xtrb = np.ascontiguousarray(
            xt_k[:, :, 768:1024].transpose(1, 0, 2).reshape(128, KT * 256))
        # wv1/wv2 [128, KT*1024]: row p = concat_k W[k*128+p, :]
        wv1 = np.ascontiguousarray(
            w_slots[1].reshape(KT, 128, D).transpose(1, 0, 2).reshape(
                128, KT * D))
        wv2 = np.ascontiguousarray(
            w_slots[2].reshape(KT, 128, D).transpose(1, 0, 2).reshape(
                128, KT * D))
        in_maps.append({"b01": b01, "bp": bp,
                        "xtra": xtra, "xtrb": xtrb,
                        "wv1": wv1, "wv2": wv2})

    nc = _get_program()
    global LAST_RESULTS
    LAST_RESULTS = run_bass_kernel_spmd(nc, in_maps, list(range(NCORES)))

    out = np.empty((T, D), np.float32)
    for c in range(NCORES):
        o_c = np.asarray(LAST_RESULTS.results[c]["o"]).astype(np.float32)
        r = 0
        for r0, n in core_rows[c]:
            out[r0:r0 + n] = o_c[r:r + n]
            r += n
    return out.astype(out_dtype, copy=False)


# revision 13
# speedup vs baseline: 1.0642x; 1.0478x over previous
"""Grouped linear (MoE routed GEMM) on 8 Trainium2 NeuronCores.

out[t] = hidden_states[t] @ weight[g(t)] where g(t) is the expert owning
token t (contiguous groups sized by tokens_per_expert).

Strategy (expert-parallel, token-balanced):
  - All group sizes are multiples of 128 -> 64 row-tiles of 128 tokens;
    each core gets exactly 8 row-tiles (1024 tokens). SPMD static slot
    pattern [0,0,0,1,1,1,2,2]: 3 weight slots per core covering 3/3/2
    row-tiles; the host decomposes the per-expert tile counts into
    sixteen 3-tile parts + eight 2-tile parts, assigns (expert ->
    core,slot), and packs per-core inputs in exact consume order.
  - All loads ride ONE HWDGE ring (scalar engine, whose framework
    preamble retires earliest) in exact consume order. Wave-0 (slot-0
    activations+weights) is packed for MAXIMUM DMA LINE SIZE: batches
    k=0,1 as single [128,1408] transfers (2816B lines) and k=2..7 as
    three [128,2816] PAIRS (5632B lines). NTFF analysis of the split
    896/512-col layout showed the wave-0 phase ran at ~230GB/s
    (1792/1024B lines are packet-rate limited) vs 400-430GB/s for
    4096B lines; big lines compress the wave-0 prefix ~4us and pull
    wv1/wv2 forward, closing the slot-1 start stall.
  - PE: 13 junk warmup matmuls (N=256 on uninitialized SBUF) bridge
    the clock-gate ramp until batch 0 lands (~9.3us); the HAM clock
    flip tracks DMA-start+3.3us, so real MMs issued before it run at
    half clock - starting them as early as possible retires more of
    them cheaply. Slot 0 k-major (6 chains, one gate per round: b0,
    b1, then one per pair on even k). Slot 1 as two 3-chain k-major
    waves paced by the wv1 chunks. Slot 2 CHAIN-major (loads finish
    ~28.5us, well before slot 2 runs, so there is no load tail to
    ride): chains 12,13,14 stop early and spread their casts+stores,
    and the last chain runs as two sequential 256-wide half-chains on
    two DIFFERENT PSUM banks so the first half's cast+store overlap
    the second half's matmuls.
  - Stores: row-tile PAIRED [128,1024] stores (2048B lines) for rt0-6
    gated on both halves' casts; rt7 stays split (512 + 256 + 256)
    for tail overlap, and the FINAL 256-wide store rides the SYNC
    engine's HWDGE ring (warmed by a tiny dummy store at program
    start) so it does not queue behind the scalar ring's previous
    store. Final quiesce (all stores landed) is mandatory: ending the
    program with DMAs in flight wedges the device.

Measured baseline of the split-layout version: 46.3us graded window,
of which ~7.5us is the fixed NKI wrapper epilogue (zeroes all 256
sems) and ~0.5us preamble tail. This layout targets ~41-42us.
"""

import os
import numpy as np
import ml_dtypes

from concourse import bacc, mybir
from concourse.bass_utils import run_bass_kernel_spmd

T, D, G, NCORES = 8192, 1024, 8, 8
TPC = T // NCORES            # tokens per core
RT = TPC // 128              # row tiles per core (8)
KT = D // 128                # contraction tiles (8)
PATTERN = (0, 0, 0, 1, 1, 1, 2, 2)   # row-tile -> weight slot
WARMUP_MMS = int(os.environ.get("K_WARMUP", "14"))

CDT = mybir.dt.bfloat16      # compute dtype on device
NP_CDT = ml_dtypes.bfloat16
ODT = mybir.dt.bfloat16      # device output dtype (host upcasts)

BW = 384 + 1024              # batch cols: [xt_k rt0-2 | w0_k]

_PROG = None
LAST_RESULTS = None          # test harness reads exec_time_ns from here


def _build_program():
    """Raw (no-Tile) program, identical on all 8 cores.

    DRAM inputs, host-packed in consume order:
      b01 [2, 128, 1408] bf16: wave-0 batches k=0,1 (xt_k rt0-2 | w0_k)
      bp  [3, 128, 2816] bf16: wave-0 batch pairs (2,3),(4,5),(6,7) -
                               5632B DMA lines
      xtra [128, KT*384] bf16: slot-1 row-tile activations (rt3-5)
      xtrb [128, KT*256] bf16: slot-2 row-tile activations (rt6-7)
      wv1, wv2 [128, KT*1024] bf16: slot-1/2 weights, k-tile k at
                               cols k*1024 (4096B lines)
    """
    nc = bacc.Bacc("TRN2", target_bir_lowering=False, debug=False,
                   num_devices=NCORES)
    b0a_d = nc.dram_tensor("b0a", [128, 896], CDT, kind="ExternalInput")
    b0b_d = nc.dram_tensor("b0b", [128, 512], CDT, kind="ExternalInput")
    b1_d = nc.dram_tensor("b1", [128, BW], CDT, kind="ExternalInput")
    bp_d = nc.dram_tensor("bp", [3, 128, 2 * BW], CDT, kind="ExternalInput")
    xtra_d = nc.dram_tensor("xtra", [128, KT * 384], CDT,
                            kind="ExternalInput")
    xtrb_d = nc.dram_tensor("xtrb", [128, KT * 256], CDT,
                            kind="ExternalInput")
    wv1_d = nc.dram_tensor("wv1", [128, KT * 1024], CDT,
                           kind="ExternalInput")
    wv2_d = nc.dram_tensor("wv2", [128, KT * 1024], CDT,
                           kind="ExternalInput")
    o_d = nc.dram_tensor("o", [TPC, D], ODT, kind="ExternalOutput")

    # batch k in SBUF: cols [k*BW, (k+1)*BW) = xt_k(rt0-2) | w0_k
    b_sb = nc.alloc_sbuf_tensor("bs", [128, KT * BW], CDT).ap()
    xtra_sb = nc.alloc_sbuf_tensor("xtras", [128, KT * 384], CDT).ap()
    xtrb_sb = nc.alloc_sbuf_tensor("xtrbs", [128, KT * 256], CDT).ap()
    wv1_sb = nc.alloc_sbuf_tensor("wv1s", [128, KT * 1024], CDT).ap()
    wv2_sb = nc.alloc_sbuf_tensor("wv2s", [128, KT * 1024], CDT).ap()
    ot_sb = nc.alloc_sbuf_tensor("ots", [128, 16 * 512], ODT).ap()
    warm_sb = nc.alloc_sbuf_tensor("warm", [128, 512], CDT).ap()
    psum = [nc.alloc_psum_tensor(f"ps{i}", [128, 512], mybir.dt.float32).ap()
            for i in range(8)]

    # Per-transfer DMA sems: a shared counting sem is unsound with
    # multiple transfers in flight on one ring.
    s_b0a = nc.alloc_semaphore("sb0a")
    s_b0b = nc.alloc_semaphore("sb0b")
    s_b1 = nc.alloc_semaphore("sb1")
    s_p = [nc.alloc_semaphore(f"sp{j}") for j in range(3)]
    s_xr = [nc.alloc_semaphore(f"sxr{j}") for j in range(2)]
    s_w1 = [nc.alloc_semaphore(f"sw1_{j}") for j in range(4)]
    s_w2 = [nc.alloc_semaphore(f"sw2_{j}") for j in range(4)]
    s_mm = nc.alloc_semaphore("smm")   # chain stop completions
    s_cp = nc.alloc_semaphore("scp")   # PSUM->SBUF cast completions
    s_st = nc.alloc_semaphore("sst")   # store completions (total count)

    # chain c = (rt, oh): rt = c//2, oh = c%2; completion order == c.
    # banks: slot-0 chains 0-5 -> 0-5; slot-1 wave A (6,7,8) -> 6,7,0;
    # wave B (9,10,11) -> 1,2,3; slot-2 (12..15) -> 4,5,6,7.
    # Warmup also uses bank 6 (in-order PE frees it before chain 6).
    bank_of = [0, 1, 2, 3, 4, 5, 6, 7, 0, 1, 2, 3, 4, 5, 6, 7]

    def xt_ap(k, rt):
        if rt < 3:
            lo = k * BW + rt * 128
            return b_sb[:, lo: lo + 128]
        if rt < 6:
            lo = k * 384 + (rt - 3) * 128
            return xtra_sb[:, lo: lo + 128]
        lo = k * 256 + (rt - 6) * 128
        return xtrb_sb[:, lo: lo + 128]

    def w_ap(s, k, oh):
        if s == 0:
            lo = k * BW + 384 + oh * 512
            return b_sb[:, lo: lo + 512]
        t = wv1_sb if s == 1 else wv2_sb
        return t[:, k * 1024 + oh * 512: k * 1024 + (oh + 1) * 512]

    # Everything is emitted into the pre-barrier main block: each
    # engine's stream is purely semaphore-driven.

    sc = nc.scalar
    # batch 0 ships in two pieces: [xt rt0-2 | w0 oh0] unblocks the
    # round-0 oh0 chains as early as the cold-ramping hose allows.
    sc.dma_start(b_sb[:, 0:896], b0a_d[:, :]).then_inc(s_b0a, 16)
    sc.dma_start(b_sb[:, 896:BW], b0b_d[:, :]).then_inc(s_b0b, 16)
    sc.dma_start(b_sb[:, BW:2 * BW], b1_d[:, :]).then_inc(s_b1, 16)
    for j in range(3):
        sc.dma_start(b_sb[:, (2 + 2 * j) * BW:(4 + 2 * j) * BW],
                     bp_d[j]).then_inc(s_p[j], 16)

    def xtra_chunk(j):
        sc.dma_start(xtra_sb[:, j * 1536:(j + 1) * 1536],
                     xtra_d[:, j * 1536:(j + 1) * 1536]).then_inc(s_xr[j], 16)

    def wv_chunk(sems, src, dst, j):
        sc.dma_start(dst[:, j * 2048:(j + 1) * 2048],
                     src[:, j * 2048:(j + 1) * 2048]).then_inc(sems[j], 16)

    # xtr chunks precede the wv chunk whose round-gate needs them, so
    # per-engine FIFO makes the wv gates cover xtr with no extra waits.
    xtra_chunk(0)
    wv_chunk(s_w1, wv1_d, wv1_sb, 0)
    wv_chunk(s_w1, wv1_d, wv1_sb, 1)
    xtra_chunk(1)
    wv_chunk(s_w1, wv1_d, wv1_sb, 2)
    wv_chunk(s_w1, wv1_d, wv1_sb, 3)
    sc.dma_start(xtrb_sb[:, 0:1024], xtrb_d[:, 0:1024]).then_inc(s_xr[0], 16)
    wv_chunk(s_w2, wv2_d, wv2_sb, 0)
    wv_chunk(s_w2, wv2_d, wv2_sb, 1)
    sc.dma_start(xtrb_sb[:, 1024:2048],
                 xtrb_d[:, 1024:2048]).then_inc(s_xr[1], 16)
    wv_chunk(s_w2, wv2_d, wv2_sb, 2)
    wv_chunk(s_w2, wv2_d, wv2_sb, 3)
    # Stores are gated behind the LAST load: the DMA hardware shares
    # read+write bandwidth across all ready descriptors in the queue
    # (NOT strict FIFO drain), so store traffic issued earlier would
    # slow the wv2 tail that paces slot-2 (measured: wv2c3 slipped
    # 30->36us with stores interleaved).
    sc.wait_ge(s_w2[3], 16)
    # Paired [128,1024] stores (2048B lines) for rt0-6: both halves'
    # casts done -> one store. rt7 split for tail overlap.
    for rt in range(7):
        sc.wait_ge(s_cp, 2 * rt + 2)
        sc.dma_start(o_d[rt * 128:(rt + 1) * 128, :],
                     ot_sb[:, rt * 1024:(rt + 1) * 1024]).then_inc(s_st, 16)
    sc.wait_ge(s_cp, 15)
    sc.dma_start(o_d[896:1024, 0:512],
                 ot_sb[:, 14 * 512:15 * 512]).then_inc(s_st, 16)
    sc.wait_ge(s_cp, 16)
    sc.dma_start(o_d[896:1024, 512:768],
                 ot_sb[:, 15 * 512:15 * 512 + 256]).then_inc(s_st, 16)
    sc.wait_ge(s_cp, 17)
    sc.dma_start(o_d[896:1024, 768:1024],
                 ot_sb[:, 15 * 512 + 256:16 * 512]).then_inc(s_st, 16)
    # NOTE: sync-engine DMAs were tried for the final store (second
    # HWDGE ring) and rejected: sync-ring participation lengthens the
    # framework preamble rendezvous by ~0.6us, shifting the WHOLE
    # schedule right for a <=0.9us tail gain.

    # -- sync: quiesce (all 10 stores landed) before the final
    # rendezvous. Required: ending the program with DMAs in flight
    # wedges the device (NRT_EXEC_UNIT_UNRECOVERABLE).
    nc.sync.wait_ge(s_st, 16 * 10)

    # -- tensor: junk warmups on uninitialized SBUF bridge the clock
    # ramp until batch 0 lands. The PSUM target is overwritten by the
    # first start=True MM of its real tenant.
    te = nc.tensor
    for _ in range(WARMUP_MMS):
        te.matmul(psum[6][:, 0:256], warm_sb[:, 0:128], warm_sb[:, 0:256],
                  start=True, stop=True)
    # slot 0: k-major; one inline gate per round on the first chain.
    # Round 0 runs the oh0 chains first so they start on the smaller
    # a-piece; the oh1 chains gate on the b-piece.
    for k in range(KT):
        order = (0, 2, 4, 1, 3, 5) if k == 0 else range(6)
        for ci in order:
            rt, oh = ci // 2, ci % 2
            mm = te.matmul(psum[ci][:], xt_ap(k, rt), w_ap(0, k, oh),
                           start=(k == 0), stop=(k == KT - 1))
            if k == 0 and ci == 0:
                mm._wait_ge(s_b0a, 16)
            elif k == 0 and ci == 1:
                mm._wait_ge(s_b0b, 16)
            elif ci == 0:
                if k == 1:
                    mm._wait_ge(s_b1, 16)
                elif k % 2 == 0:
                    mm._wait_ge(s_p[k // 2 - 1], 16)
            if k == KT - 1:
                mm.then_inc(s_mm)
    # slot 1: two k-major waves of 3 chains (only banks 6,7,0 resp.
    # 1,2,3 are free in time); paced by the wv1 chunks.
    for wave, chains in ((0, (6, 7, 8)), (1, (9, 10, 11))):
        for k in range(KT):
            for c in chains:
                rt, oh = c // 2, c % 2
                mm = te.matmul(psum[bank_of[c]][:], xt_ap(k, rt),
                               w_ap(1, k, oh),
                               start=(k == 0), stop=(k == KT - 1))
                if wave == 0 and k % 2 == 0 and c == 6:
                    mm._wait_ge(s_w1[k // 2], 16)
                if k == 0 and c >= 8:
                    # bank reused: prior tenant's cast done
                    mm._wait_ge(s_cp, c - 7)
                if k == KT - 1:
                    mm.then_inc(s_mm)
    # slot 2 chain-major: loads finish well before slot 2 runs, so
    # chains 12,13,14 stop early and spread their casts+stores across
    # the remaining compute. Chain 12 carries the wv2 chunk gates
    # (in-order PE covers chains 13,14).
    for c in (12, 13, 14):
        rt, oh = c // 2, c % 2
        for k in range(KT):
            if c == 12 and k % 2 == 0:
                te.wait_ge(s_w2[k // 2], 16)
            mm = te.matmul(psum[bank_of[c]][:], xt_ap(k, rt),
                           w_ap(2, k, oh),
                           start=(k == 0), stop=(k == KT - 1))
            if k == 0:
                mm._wait_ge(s_cp, c - 7)      # bank free
            if k == KT - 1:
                mm.then_inc(s_mm)
    # The very last chain runs as two sequential 256-wide half-chains
    # on two DIFFERENT banks (7 then 0, both long free), so the first
    # half's cast+store overlap the second half's matmuls. (Same-bank
    # splitting crashes: PE-write + DVE-read of one bank is illegal.)
    for h, bank in ((0, 7), (1, 0)):
        for k in range(KT):
            lo = k * 1024 + 512 + h * 256
            mm = te.matmul(psum[bank][:, 0:256],
                           xt_ap(k, 7), wv2_sb[:, lo: lo + 256],
                           start=(k == 0), stop=(k == KT - 1))
            if k == 0:
                mm._wait_ge(s_cp, 8 + h)      # bank free (chain 7 / 8)
            if k == KT - 1:
                mm.then_inc(s_mm)

    # -- vector: PSUM->SBUF casts in chain-completion order
    for c in range(15):
        cp = nc.vector.tensor_copy(ot_sb[:, c * 512:(c + 1) * 512],
                                   psum[bank_of[c]][:])
        cp._wait_ge(s_mm, c + 1)
        cp.then_inc(s_cp)
    for h, bank in ((0, 7), (1, 0)):
        lo = 15 * 512 + h * 256
        cp = nc.vector.tensor_copy(ot_sb[:, lo: lo + 256],
                                   psum[bank][:, 0:256])
        cp._wait_ge(s_mm, 16 + h)
        cp.then_inc(s_cp)

    with nc.Block():
        pass

    nc.compile()
    return nc


def _get_program():
    global _PROG
    if _PROG is None:
        _PROG = _build_program()
    return _PROG


def _solve_parts(tiles_per_expert):
    """Decompose per-expert tile counts into 16 parts of 3 tiles and 8
    parts of 2 tiles. Returns (threes, twos) as lists of expert ids, or
    None if infeasible."""
    t = list(tiles_per_expert)
    f = [c % 2 for c in t]              # number of 3-parts per expert
    if any(3 * f[g] > t[g] for g in range(len(t))):
        return None
    h = [(t[g] - 3 * f[g]) // 2 for g in range(len(t))]
    # each f+=2 converts three 2-parts into two 3-parts
    while sum(h) > 8:
        g = max(range(len(t)), key=lambda i: h[i])
        if h[g] < 3:
            return None
        f[g] += 2
        h[g] -= 3
    if sum(h) != 8 or sum(f) != 16:
        return None
    threes, twos = [], []
    for g in range(len(t)):
        threes += [g] * f[g]
        twos += [g] * h[g]
    return threes, twos


def _numpy_fallback(hidden_states, weight, counts):
    out = np.empty((hidden_states.shape[0], weight.shape[2]), np.float32)
    start = 0
    for g in range(weight.shape[0]):
        end = start + int(counts[g])
        out[start:end] = hidden_states[start:end].astype(np.float32) @ \
            weight[g].astype(np.float32)
        start = end
    return out


def kernel(hidden_states, weight, tokens_per_expert):
    counts = np.asarray(tokens_per_expert).astype(np.int64)
    out_dtype = hidden_states.dtype

    ok = (hidden_states.shape == (T, D) and weight.shape == (G, D, D)
          and counts.shape == (G,) and counts.sum() == T
          and np.all(counts % 128 == 0) and np.all(counts >= 0))
    parts = _solve_parts(counts // 128) if ok else None
    if parts is None:
        return _numpy_fallback(hidden_states, weight, counts).astype(out_dtype)
    threes, twos = parts

    # Global preprocessing: transpose+cast activations once, cast weights.
    ht = np.ascontiguousarray(
        np.asarray(hidden_states, dtype=np.float32).astype(NP_CDT).T)
    wc = np.asarray(weight, dtype=np.float32).astype(NP_CDT)  # [G, D, D]

    # Per-expert global row offsets; consume tiles in order.
    expert_row = dict(
        (g, int(o)) for g, o in enumerate(np.concatenate(
            [[0], np.cumsum(counts)[:-1]])))

    in_maps = []
    core_rows = []       # per core: list of (global_row_start, n_rows)
    for c in range(NCORES):
        part_list = [(threes[2 * c], 3 * 128), (threes[2 * c + 1], 3 * 128),
                     (twos[c], 2 * 128)]
        spans = []
        for g, nrows in part_list:
            r0 = expert_row[g]
            expert_row[g] = r0 + nrows
            spans.append((r0, nrows))
        core_rows.append(spans)
        # xt_c: [D, TPC] activations (pre-transposed); k-tile k = rows
        # k*128..k*128+127.
        xt_c = np.concatenate(
            [ht[:, r0:r0 + n] for r0, n in spans], axis=1)
        w_slots = [wc[g] for g, _ in part_list]   # 3 x [D, D] bf16

        # wave-0 batch k, partition-major: batch[k, p] =
        # xt[k*128+p, 0:384] | w0[k*128+p, :]  -> [KT, 128, 1408]
        xt_k = xt_c.reshape(KT, 128, TPC)
        w0_k = w_slots[0].reshape(KT, 128, D)
        wv0 = np.empty((KT, 128, BW), dtype=NP_CDT)
        wv0[:, :, 0:384] = xt_k[:, :, 0:384]
        wv0[:, :, 384:BW] = w0_k
        b0a = np.ascontiguousarray(wv0[0][:, 0:896])
        b0b = np.ascontiguousarray(wv0[0][:, 896:BW])
        b1 = np.ascontiguousarray(wv0[1])
        # pairs (2,3),(4,5),(6,7): row p = [batch2j(p,:)|batch2j+1(p,:)]
        bp = np.ascontiguousarray(
            wv0[2:].reshape(3, 2, 128, BW).transpose(0, 2, 1, 3).reshape(
                3, 128, 2 * BW))
        xtra = np.ascontiguousarray(
            xt_k[:, :, 384:768].transpose(1, 0, 2).reshape(128, KT * 384))
        xtrb = np.ascontiguousarray(
            xt_k[:, :, 768:1024].transpose(1, 0, 2).reshape(128, KT * 256))
        # wv1/wv2 [128, KT*1024]: row p = concat_k W[k*128+p, :]
        wv1 = np.ascontiguousarray(
            w_slots[1].reshape(KT, 128, D).transpose(1, 0, 2).reshape(
                128, KT * D))
        wv2 = np.ascontiguousarray(
            w_slots[2].reshape(KT, 128, D).transpose(1, 0, 2).reshape(
                128, KT * D))
        in_maps.append({"b0a": b0a, "b0b": b0b, "b1": b1, "bp": bp,
                        "xtra": xtra, "xtrb": xtrb,
                        "wv1": wv1, "wv2": wv2})

    nc = _get_program()
    global LAST_RESULTS
    LAST_RESULTS = run_bass_kernel_spmd(nc, in_maps, list(range(NCORES)))

    out = np.empty((T, D), np.float32)
    for c in range(NCORES):
        o_c = np.asarray(LAST_RESULTS.results[c]["o"]).astype(np.float32)
        r = 0
        for r0, n in core_rows[c]:
            out[r0:r0 + n] = o_c[r:r + n]
            r += n
    return out.astype(out_dtype, copy=False)
